# revision 45
# baseline (speedup 1.0000x reference)
"""Trainium2 Bass kernel for nn_BioSimulator (phosphene pooling model).

Math: the reference materializes dist2/gauss of shape (1, 1024, 256, 256) and
reduces over the 1024 electrodes.  dist2 is separable:
    dist2[n,h,w] = ((px[w]-vx[n])*s)^2 + ((py[h]-vy[n])*s)^2
so   gauss[n,h,w] = gx[n,w] * gy[n,h]   with
    gx[n,w] = exp(-((px[w]-vx[n])*s*rs_n)^2),  rs_n = 1/(sqrt(2)*sigma_n)
and  out[h,w]  = sum_n Bamp[n] * gy[n,h] * gx[n,w]  — a (H x N) @ (N x W)
matmul with K = 1024.  The per-electrode model configuration (wedge-dipole
retinotopy via complex exp/div, sigma, Bamp) is computed on-chip on [128, 8]
tiles (electrode n = 128*j + p: partition p, chunk column j).

Raw bacc (no TileContext): explicit semaphores, which drops the Tile
drain + EVSEM-butterfly epilogue (~10 us).  Same-engine RAW chains need no
sems (the engine pipe drains between consecutive ops); cross-engine deps use
four counting semaphores with transitive waits.  The epilogue clears the
semaphores (sequenced by a plain s_dma wait) so the NEFF can re-execute.

ACT-table discipline: the scalar engine reloads its lookup table (~1.3 us)
whenever the activation function leaves the loaded set, so this kernel only
uses EXP and LN (which share the natural_log_exp_and_others set) plus the
table-free SQUARE.  sin/cos are degree-7/6 polynomials on the vector engine,
sqrt(x) = exp(0.5*ln(x)), and sigmoid = 1/(1 + e^sh * exp(-slope*q)) via DVE
reciprocal.  One table load total.

Sharding: 2x4 grid over the output — core c computes the h-half hh = c // 4
(128 rows) and w-quarter wq = c % 4 (64 cols).  Every core evaluates all 1024
electrodes for its slice (fully local, no collectives); the host stitches the
8 [128, 64] slices into the (1, 1, 256, 256) output.
"""

import numpy as np

GRID = 32
OUT = 256
FOV = 30.0
N_CORES = 8
NCHUNK = 8  # 1024 electrodes / 128 partitions

K_, A_, B_ = 17.3, 0.75, 120.0
SLOPE, HALF, RHEO = 19152642.5, 1.057e-07, 2.39e-05
FREQ, PW, R2S = 300.0, 0.00017, 0.5
DEG2PIX = OUT / (2.0 * FOV)
DEG2RAD = float(np.pi / 180.0)
INVK = 1.0 / K_
AB = A_ * B_
SLP = SLOPE * PW * FREQ            # 976784.7675
ESH = float(np.exp(SLOPE * HALF))  # e^{slope*half}
SQRT2 = float(np.sqrt(2.0))

# sin(x) = x * P(x^2), cos(x) = Q(x^2); least-squares fits on |x| <= 0.9,
# max abs error ~2e-7 in fp32 (used for the gyn/k rotation angle)
SIN_C = (0.999999993645295, -0.1666663839873324, 0.008331410967920568,
         -0.00019428598847529545)
COS_C = (0.9999999430059742, -0.49999746415333846, 0.041649415317051235,
         -0.0013518287615003882)

# packed input column layout: [stim | pp | gxe | gye | pxs | pys]
C_STIM, C_PP, C_GXE, C_GYE, C_PXS, C_PYS, C_END = 0, 8, 21, 29, 37, 101, 229

_CACHE: dict = {}


def _host_constants():
    """Electrode / pixel grids (input-independent)."""
    if "consts" in _CACHE:
        return _CACHE["consts"]
    xc = np.linspace(-15.0, 15.0, GRID, dtype=np.float32)
    gx, gy = np.meshgrid(xc, xc, indexing="xy")
    # electrode n = 128*j + p  ->  [128, 8] with [p, j] = flat[j*128 + p]
    gxe = gx.reshape(-1).astype(np.float32).reshape(NCHUNK, 128).T.copy()
    gye = gy.reshape(-1).astype(np.float32).reshape(NCHUNK, 128).T.copy()
    xs = np.linspace(-FOV, FOV, OUT, dtype=np.float32)
    _CACHE["consts"] = (gxe, gye, xs)
    return _CACHE["consts"]


def _build_nc(self_waits=False):
    """Build the SPMD raw-bacc program (same program on all 8 cores).

    self_waits=True adds a same-engine retire-wait to every DVE/ACT op so the
    CoreSim race detector can fully validate the cross-engine semaphores (it
    does not model the engines' own pipe-drain between consecutive ops, which
    makes same-engine RAW safe on silicon).  The hardware build omits them:
    they cost ~100 ns of sem latency per op.
    """
    key = ("nc", self_waits)
    if key in _CACHE:
        return _CACHE[key]

    import concourse.bacc as bacc
    import concourse.mybir as mybir

    f32 = mybir.dt.float32
    AF = mybir.ActivationFunctionType
    OP = mybir.AluOpType

    # Table-set override: the stock insert_act_table_loads maps exp -> the
    # exp_and_others set and ln -> natural_log, which thrashes the ACT table
    # (1.3 us per reload) on our exp/ln/exp sequence.  The act_func_set_id is
    # the list INDEX into act_info.json, so the list order must be preserved;
    # strip our functions from every other set instead, which leaves
    # natural_log_exp_and_others as the only candidate -> one table load.
    class _Bacc(bacc.Bacc):
        def insert_act_table_loads(self):
            from concourse.hw_specs import get_activation_tables
            from concourse import bacc as _bacc_mod

            has_activation = any(
                isinstance(i, mybir.InstActivation)
                for b in self.main_func.blocks
                for i in b.instructions
            )
            if not has_activation:
                return
            tabs = get_activation_tables(self.m.arch)
            pref = "natural_log_exp_and_others"
            ours = {AF.Exp, AF.Ln, AF.Square, AF.Copy, AF.Relu, AF.Identity}
            tables = [
                (k, (v if k == pref else (v - ours))) for k, v in tabs.items()
            ]
            _bacc_mod._bass_rust.insert_act_table_loads(self, tables)

    nc = _Bacc(None, detect_race_conditions=self_waits)
    d_inp = nc.declare_dram_parameter("inp", [128, C_END], f32, isOutput=False)
    d_o = nc.declare_dram_parameter("o", [128, 64], f32, isOutput=True)

    V, S, P, SY, G = nc.vector, nc.scalar, nc.tensor, nc.sync, nc.gpsimd

    def sb(name, w):
        return nc.alloc_sbuf_tensor(name, [128, w], f32)

    inp = sb("inpt", C_END)
    stim = inp[:, C_STIM:C_STIM + 8]
    gxe = inp[:, C_GXE:C_GXE + 8]
    gye = inp[:, C_GYE:C_GYE + 8]
    pxs = inp[:, C_PXS:C_PXS + 64]
    pys = inp[:, C_PYS:C_PYS + 128]

    def ppc(i):  # patient_params column i as [128, 1]
        return inp[:, C_PP + i:C_PP + i + 1]

    names = ["th", "qt", "ct", "stp", "st", "dxs", "dys", "irho", "t1", "t2",
             "gxn", "t3", "t4", "gyn", "ang", "qa", "sp", "si", "co", "er",
             "ewr", "ewi", "nr", "ni", "dr", "di", "den", "t5", "iden", "q1",
             "q2", "zr", "q3", "q4", "zi", "t6", "t7", "mk", "me", "uu", "vv",
             "sg", "rsd", "rs", "nrs", "nvx", "nvy", "tie", "ie", "exm", "u1",
             "bamp"]
    t = {n: sb(n, 8) for n in names}
    pk = sb("pk", 16)     # [r^2 | stim*irho*8e-5] for the packed sqrt
    lnp = sb("lnp", 16)
    rsb = sb("rsb", 16)
    pr2 = sb("pr2", 16)   # [r+a | r+b] for the packed reciprocal
    irab = sb("irab", 16)
    dxt = [sb(f"dx{j}", 64) for j in range(NCHUNK)]
    dyt = [sb(f"dy{j}", 128) for j in range(NCHUNK)]
    sqt = [sb(f"sq{j}", 192) for j in range(NCHUNK)]
    gpt = [sb(f"gpt{j}", 192) for j in range(NCHUNK)]
    gxb = [sb(f"gxb{j}", 64) for j in range(NCHUNK)]
    ot = sb("ot", 64)
    e1 = sb("e1", 64)
    e2 = sb("e2", 64)
    o2 = sb("o2", 64)
    e3 = sb("e3", 64)
    ob = sb("ob", 64)
    acc = nc.alloc_psum_tensor("accp", [128, 64], f32)

    s_dma = nc.alloc_semaphore("s_dma")
    s_dm2 = nc.alloc_semaphore("s_dm2")
    s_dve = nc.alloc_semaphore("s_dve")
    s_act = nc.alloc_semaphore("s_act")
    s_pe = nc.alloc_semaphore("s_pe")

    nd = [0]
    na = [0]
    wt: dict = {}  # tensor name -> s_dve tick of its last DVE write

    def _nm(x):
        try:
            return x.tensor.name
        except AttributeError:
            return None

    # DVE same-engine RAW needs a sem on silicon (verified by bisection: the
    # no-wait build returns wrong values in the gaussian region, the
    # DVE-self-wait build is exact).  Dep-tracked wait values make the wait
    # free whenever the producer is a few slots back (its sem has already
    # posted); the emission below interleaves independent chains so direct
    # producer->consumer neighbors are rare.  ACT->ACT chains were verified
    # safe without sems (and on the Tile build as well).
    # self_waits=True upgrades to blanket retire-waits for the CoreSim race
    # detector's benefit (it also flags ACT->ACT).
    def dve(inst, outs, ins):
        if self_waits in (True, "dve") and nd[0] > 0:
            inst._wait_ge(s_dve, nd[0])
        else:
            need = 0
            for x in ins:
                nm = _nm(x)
                if nm is not None:
                    need = max(need, wt.get(nm, 0))
            # producers >= 8 ops back have retired: the queue is 8 deep and
            # execution is in-order, so their writeback long since completed
            if need > 0 and nd[0] - need < 8:
                inst._wait_ge(s_dve, need)
        inst.then_inc(s_dve, 1)
        nd[0] += 1
        for x in outs:
            nm = _nm(x)
            if nm is not None:
                wt[nm] = nd[0]
        return nd[0]

    def acti(inst):
        if self_waits in (True, "act") and na[0] > 0:
            inst._wait_ge(s_act, na[0])
        inst.then_inc(s_act, 1)
        na[0] += 1
        return na[0]

    def ts(out, in0, s1, s2, op0, op1=None):
        if op1 is None:
            inst = V.tensor_scalar(out, in0, s1, None, op0)
        else:
            inst = V.tensor_scalar(out, in0, s1, s2, op0, op1)
        return dve(inst, [out], [in0, s1, s2])

    def tt(out, in0, in1, op):
        return dve(V.tensor_tensor(out, in0, in1, op), [out], [in0, in1])

    def stt(out, in0, s, in1, op0, op1):
        return dve(
            V.scalar_tensor_tensor(out, in0, s, in1, op0, op1),
            [out], [in0, s, in1],
        )

    def rcp(out, in0):
        return dve(V.reciprocal(out, in0), [out], [in0])

    # ================= program =================
    # split input DMA: the 37 param columns gate the DVE chain and land
    # first; the 192 pixel-grid columns are only needed by the chunk loop
    SY.dma_start(out=inp[:, 0:C_PXS], in_=d_inp[:, 0:C_PXS]).then_inc(
        s_dma, 16)
    SY.dma_start(out=inp[:, C_PXS:C_END], in_=d_inp[:, C_PXS:C_END]).then_inc(
        s_dm2, 16)

    # ---- DVE: params (stim-only path first so ACT can start early) ----
    V.wait_ge(s_dma, 16)
    ts(t["tie"][:], stim, 8e-05, -RHEO, OP.mult, OP.add)
    m_ie = ts(t["ie"][:], t["tie"][:], 0.0, None, OP.max)
    rho9 = sb("rho9", 1)
    ts(rho9[:], ppc(0), 1.0, 1e-09, OP.mult, OP.add)
    rcp(t["irho"][:, 0:1], rho9[:])
    m_sb2 = ts(pk[:, 8:16], stim, t["irho"][:, 0:1], 8e-05, OP.mult, OP.mult)

    th, qt, ct, stp, st = t["th"], t["qt"], t["ct"], t["stp"], t["st"]
    ts(th[:, 0:1], ppc(12), DEG2RAD, None, OP.mult)
    tt(qt[:, 0:1], th[:, 0:1], th[:, 0:1], OP.mult)
    ts(ct[:, 0:1], qt[:, 0:1], -0.5, 1.0, OP.mult, OP.add)
    ts(stp[:, 0:1], qt[:, 0:1], -1.0 / 6.0, 1.0, OP.mult, OP.add)
    ts(t["dxs"][:, 0:1], ppc(10), 1.0 / 300.0, None, OP.mult)
    ts(t["dys"][:, 0:1], ppc(11), 1.0 / 300.0, None, OP.mult)
    tt(st[:, 0:1], th[:, 0:1], stp[:, 0:1], OP.mult)

    ts(t["t1"][:], gxe, ct[:, 0:1], None, OP.mult)
    stt(t["t2"][:], gye, st[:, 0:1], t["t1"][:], OP.mult, OP.subtract)
    m_gxn = ts(t["gxn"][:], t["t2"][:], -1.0, t["dxs"][:, 0:1], OP.mult, OP.add)
    ts(t["t3"][:], gxe, st[:, 0:1], None, OP.mult)
    stt(t["t4"][:], gye, ct[:, 0:1], t["t3"][:], OP.mult, OP.add)
    ts(t["gyn"][:], t["t4"][:], 1.0, t["dys"][:, 0:1], OP.mult, OP.add)

    ang, qa, sp, si, co = t["ang"], t["qa"], t["sp"], t["si"], t["co"]
    ts(ang[:], t["gyn"][:], INVK, None, OP.mult)
    tt(qa[:], ang[:], ang[:], OP.mult)
    ts(sp[:], qa[:], SIN_C[3], SIN_C[2], OP.mult, OP.add)
    tt(sp[:], sp[:], qa[:], OP.mult)
    ts(sp[:], sp[:], SIN_C[1], None, OP.add)
    tt(sp[:], sp[:], qa[:], OP.mult)
    ts(sp[:], sp[:], SIN_C[0], None, OP.add)
    tt(si[:], sp[:], ang[:], OP.mult)
    ts(co[:], qa[:], COS_C[3], COS_C[2], OP.mult, OP.add)
    tt(co[:], co[:], qa[:], OP.mult)
    ts(co[:], co[:], COS_C[1], None, OP.add)
    tt(co[:], co[:], qa[:], OP.mult)
    ts(co[:], co[:], COS_C[0], None, OP.add)

    # ---- ACT: exm and er (order matches rising DVE ticks) ----
    S.wait_ge(s_dve, m_ie)
    m_exm = acti(S.activation(t["exm"][:], t["ie"][:], AF.Exp, scale=-SLP))
    S.wait_ge(s_dve, m_gxn)
    m_er = acti(S.activation(t["er"][:], t["gxn"][:], AF.Exp, scale=INVK))

    # ---- DVE: complex z = a*b*(ew-1)/(b - a*ew) ----
    V.wait_ge(s_act, m_er)
    tt(t["ewr"][:], t["er"][:], co[:], OP.mult)
    tt(t["ewi"][:], t["er"][:], si[:], OP.mult)
    ts(t["nr"][:], t["ewr"][:], AB, -AB, OP.mult, OP.add)
    ts(t["ni"][:], t["ewi"][:], AB, None, OP.mult)
    ts(t["dr"][:], t["ewr"][:], -A_, B_, OP.mult, OP.add)
    ts(t["di"][:], t["ewi"][:], -A_, None, OP.mult)
    tt(t["den"][:], t["dr"][:], t["dr"][:], OP.mult)
    tt(t["t5"][:], t["di"][:], t["di"][:], OP.mult)
    tt(t["den"][:], t["den"][:], t["t5"][:], OP.add)
    rcp(t["iden"][:], t["den"][:])
    tt(t["q1"][:], t["nr"][:], t["dr"][:], OP.mult)
    tt(t["q2"][:], t["ni"][:], t["di"][:], OP.mult)
    tt(t["q1"][:], t["q1"][:], t["q2"][:], OP.add)
    tt(t["zr"][:], t["q1"][:], t["iden"][:], OP.mult)
    tt(t["q3"][:], t["ni"][:], t["dr"][:], OP.mult)
    tt(t["q4"][:], t["nr"][:], t["di"][:], OP.mult)
    tt(t["q3"][:], t["q3"][:], t["q4"][:], OP.subtract)
    tt(t["zi"][:], t["q3"][:], t["iden"][:], OP.mult)
    tt(t["t6"][:], t["zr"][:], t["zr"][:], OP.mult)
    tt(t["t7"][:], t["zi"][:], t["zi"][:], OP.mult)
    m_pk = tt(pk[:, 0:8], t["t6"][:], t["t7"][:], OP.add)

    # Bamp = 1 / (1 + e^sh * exp(-slp*ie))   (DVE part)
    V.wait_ge(s_act, m_exm)
    ts(t["u1"][:], t["exm"][:], ESH, 1.0, OP.mult, OP.add)
    rcp(t["bamp"][:], t["u1"][:])

    # ---- ACT: packed sqrt of [r^2 | sb^2] via exp(0.5 ln x) ----
    S.wait_ge(s_dve, m_pk)
    acti(S.activation(lnp[:], pk[:], AF.Ln))
    m_rsb = acti(S.activation(rsb[:], lnp[:], AF.Exp, scale=0.5))
    rr = rsb[:, 0:8]
    sbase = rsb[:, 8:16]

    # ---- DVE: M, sigma, rs, centers ----
    V.wait_ge(s_act, m_rsb)
    ts(pr2[:, 0:8], rr, A_, None, OP.add)
    ts(pr2[:, 8:16], rr, B_, None, OP.add)
    rcp(irab[:], pr2[:])
    tt(t["mk"][:], irab[:, 0:8], irab[:, 8:16], OP.subtract)
    ts(t["me"][:], t["mk"][:], K_, 1e-09, OP.mult, OP.add)
    rcp(t["uu"][:], t["me"][:])
    tt(t["vv"][:], sbase, t["uu"][:], OP.mult)
    ts(t["sg"][:], t["vv"][:], R2S * DEG2PIX, 0.5, OP.mult, OP.max)
    ts(t["rsd"][:], t["sg"][:], float(np.sqrt(2.0)), None, OP.mult)
    rcp(t["rs"][:], t["rsd"][:])
    # centers: dx = pxs*rs + nvx with pxs = px*deg2pix  ->  nvx = -deg2pix*rs*v
    ts(t["nrs"][:], t["rs"][:], -DEG2PIX, None, OP.mult)
    tt(t["nvx"][:], t["zr"][:], t["nrs"][:], OP.mult)
    m_nvy = tt(t["nvy"][:], t["zi"][:], t["nrs"][:], OP.mult)

    # ---- loop: squares (DVE x / ACT y), packed EXP, Bamp fold, matmul ----
    rs, nvx, nvy, bamp = t["rs"], t["nvx"], t["nvy"], t["bamp"]
    m_sqx = [0] * NCHUNK
    m_sqy = [0] * NCHUNK
    m_exp = [0] * NCHUNK
    m_gxb = [0] * NCHUNK
    DVE_Y = tuple(j for j in range(NCHUNK) if j % 2 == 1)
    # precompute the ACT stream tick of EXP_j (4 ops precede the loop; even
    # chunks add SQUARE+EXP, odd chunks only EXP) — emit_gxb needs it before
    # the ACT stream is emitted
    _tick = 4
    for _j in range(NCHUNK):
        _tick += 1 if _j in DVE_Y else 2
        m_exp[_j] = _tick

    def emit_dve_chunk(j):
        jc = slice(j, j + 1)
        ts(dxt[j][:], pxs, rs[:, jc], nvx[:, jc], OP.mult, OP.add)
        m_sqx[j] = tt(sqt[j][:, 0:64], dxt[j][:], dxt[j][:], OP.mult)

    emit_dve_chunk(0)
    emit_dve_chunk(1)
    for j in range(NCHUNK):
        if j + 2 < NCHUNK:
            emit_dve_chunk(j + 2)
        V.wait_ge(s_act, 4 + 2 * (j + 1))  # EXP_j done
        m_gxb[j] = ts(gxb[j][:], gpt[j][:, 0:64], bamp[:, j:j + 1], None,
                      OP.mult)

    # ACT loop stream
    S.wait_ge(s_dm2, 16)  # pys columns loaded
    S.wait_ge(s_dve, m_nvy)
    for j in range(NCHUNK):
        jc = slice(j, j + 1)
        acti(S.activation(sqt[j][:, 64:192], pys, AF.Square,
                          scale=rs[:, jc], bias=nvy[:, jc]))
        S.wait_ge(s_dve, m_sqx[j])
        m_exp[j] = acti(S.activation(gpt[j][:], sqt[j][:], AF.Exp, scale=-1.0))

    # PE stream
    for j in range(NCHUNK):
        P.wait_ge(s_dve, m_gxb[j])
        P.matmul(acc[:], gpt[j][:, 64:192], gxb[j][:],
                 start=(j == 0), stop=(j == NCHUNK - 1)).then_inc(s_pe, 1)

    # ---- DVE: polynomial + clip (Estrin), then DMA out ----
    V.wait_ge(s_pe, NCHUNK)
    a0, a1, a2, a3, a4 = (ppc(3 + i) for i in range(5))
    dve(V.tensor_copy(ot[:], acc[:]), [ot[:]], [acc[:]])
    ts(e1[:], acc[:], a1, a0, OP.mult, OP.add)
    ts(e2[:], acc[:], a3, a2, OP.mult, OP.add)
    tt(o2[:], ot[:], acc[:], OP.mult)
    stt(e3[:], o2[:], a4, e2[:], OP.mult, OP.add)
    tt(e3[:], o2[:], e3[:], OP.mult)
    tt(e3[:], e3[:], e1[:], OP.add)
    # clip + DMA in two column halves: the first half's DMA flies while
    # the second half clips
    m_ob1 = ts(ob[:, 0:32], e3[:, 0:32], 0.0, 1.0, OP.max, OP.min)
    m_ob2 = ts(ob[:, 32:64], e3[:, 32:64], 0.0, 1.0, OP.max, OP.min)
    SY.wait_ge(s_dve, m_ob1)
    SY.dma_start(out=d_o[:, 0:32], in_=ob[:, 0:32]).then_inc(s_dma, 16)
    SY.wait_ge(s_dve, m_ob2)
    SY.dma_start(out=d_o[:, 32:64], in_=ob[:, 32:64]).then_inc(s_dma, 16)

    # ---- epilogue: restore sem state for NEFF re-execution.  gpsimd waits
    # on every sem's final value: each wait happens-after that sem's last
    # update, and every engine's trailing instruction is one of those
    # updates (V: ob clip -> s_dve; S: EXP_7 -> s_act, consumed by gxb7
    # before m_ob; P: matmul_7 -> s_pe, consumed by the poly; SY: the output
    # DMA -> s_dma).  So after the four waits all queues are quiesced and
    # the clears cannot race — no all-engine barrier needed (~7 us saved).
    G.wait_ge(s_dma, 48)
    G.wait_ge(s_dm2, 16)
    G.wait_ge(s_dve, nd[0])
    G.wait_ge(s_act, na[0])
    G.wait_ge(s_pe, NCHUNK)
    if self_waits:
        # the race detector only accepts sem clears after a full barrier
        nc.all_engine_barrier()
    G.sem_clear(s_dma)
    G.sem_clear(s_dm2)
    G.sem_clear(s_dve)
    G.sem_clear(s_act)
    G.sem_clear(s_pe)

    nc.finalize()
    _CACHE[key] = nc
    return nc


def _prep_in_maps(stim_np: np.ndarray, pp_np: np.ndarray):
    gxe, gye, xs = _host_constants()
    inp_base = np.empty((128, C_END), dtype=np.float32)
    inp_base[:, C_STIM:C_STIM + 8] = (
        stim_np.reshape(-1).astype(np.float32).reshape(NCHUNK, 128).T
    )
    inp_base[:, C_PP:C_PP + 13] = pp_np.reshape(1, 13).astype(np.float32)
    inp_base[:, C_GXE:C_GXE + 8] = gxe
    inp_base[:, C_GYE:C_GYE + 8] = gye
    in_maps = []
    for c in range(N_CORES):
        hh, wq = c // 4, c % 4
        inp = inp_base.copy()
        inp[:, C_PXS:C_PXS + 64] = xs[64 * wq:64 * wq + 64][None, :] * DEG2PIX
        inp[:, C_PYS:C_PYS + 128] = (
            xs[128 * hh:128 * hh + 128][None, :] * DEG2PIX
        )
        in_maps.append({"inp": inp})
    return in_maps


def _assemble(results) -> np.ndarray:
    out = np.empty((OUT, OUT), dtype=np.float32)
    for c in range(N_CORES):
        hh, wq = c // 4, c % 4
        out[128 * hh:128 * hh + 128, 64 * wq:64 * wq + 64] = results[c]["o"]
    return out.reshape(1, 1, OUT, OUT)


def kernel(stimulation: np.ndarray, patient_params: np.ndarray) -> np.ndarray:
    from concourse.bass_utils import run_bass_kernel_spmd

    stim_np = np.asarray(stimulation, dtype=np.float32)
    pp_np = np.asarray(patient_params, dtype=np.float32)
    nc = _build_nc()
    in_maps = _prep_in_maps(stim_np, pp_np)
    res = run_bass_kernel_spmd(nc, in_maps, list(range(N_CORES)))
    return _assemble(res.results)
